# revision 17
# baseline (speedup 1.0000x reference)
"""Trainium2 Bass kernel for nn_Network_61658550501610 (Mamba block + MLP head).

Reference computation (per batch element b, sequence length L=2048):
  xz = x @ W_in.T; xi, z = split(xz)
  xc = silu(causal_depthwise_conv(xi, conv_w) + conv_b)
  x_dbl = xc @ W_xproj.T -> (dt, B, C)
  delta = softplus(dt @ W_dt.T + b_dt)
  h_t = exp(delta*A)*h_{t-1} + delta*B*xc   (selective scan, state [82,16])
  y = (h @ C) + D*xc; y *= silu(z)
  out = y @ W_out.T;  logits = relu(out@W_c1.T+b_c1)@W_c2.T + b_c2

Sharding: data-parallel over batch (B=16 -> 2 per core across 8 cores).

Layout on chip: d_inner (82) on partitions, time on free dim. The scan uses
the DVE tensor_tensor_scan instruction per state index n (16 of them), with
chunk carries through per-partition initial values. B[n,:]/C[n,:] are
broadcast across partitions with TensorE ones-matmuls into PSUM; the sum
over n runs as accumulating identity matmuls on TensorE.
"""
import numpy as np

import concourse.bacc as bacc
import concourse.tile as tile
import concourse.mybir as mybir
from concourse.bass_utils import run_bass_kernel_spmd

F32 = mybir.dt.float32
BF16 = mybir.dt.bfloat16
OP = mybir.AluOpType
ACTF = mybir.ActivationFunctionType
AX = mybir.AxisListType

# problem dims (hardcoded per contract)
B, L, DM = 16, 2048, 41
DIN, N, K = 82, 16, 4          # d_inner, d_state, d_conv
DTR, HID, NL = 3, 64, 10
NCORES = 8
BLOC = B // NCORES             # batch per core

C = 512                        # time-chunk length
NCH = L // C                   # chunks per batch element
Q = C // 128                   # 128-row subtiles per chunk

_cache = {}


def _build(cfg):
    nc = bacc.Bacc("TRN2", target_bir_lowering=False, debug=False,
                   enable_asserts=False)

    def din(name, shape):
        return nc.dram_tensor(name, list(shape), F32, kind="ExternalInput").ap()

    x_d = din("x", (BLOC, L, DM))
    w_inT_d = din("w_inT", (DM, 2 * DIN))
    w_effT_d = din("w_effT", (DIN, DIN))
    w_bcT_d = din("w_bcT", (DIN, 2 * N))
    a_cols_d = din("a_cols", (DIN, N))
    conv_w_d = din("conv_w", (DIN, K))
    conv_b_d = din("conv_b", (DIN, 1))
    conv_bh_d = din("conv_bh", (DIN, 1))
    b_dt_d = din("b_dt", (DIN, 1))
    d_col_d = din("d_col", (DIN, 1))
    w1T_d = din("w1T", (DIN, HID))
    b_c1_d = din("b_c1", (HID, 1))
    w2T_d = din("w2T", (HID + 1, NL))
    ident_d = din("ident", (128, 128))
    e_sel_d = din("e_sel", (2 * N, 2 * N * DIN))
    out_d = nc.dram_tensor("out", [BLOC, L, NL], F32, kind="ExternalOutput").ap()

    with tile.TileContext(nc) as tc, tc.tile_pool(name="wts", bufs=1) as wp, \
         tc.tile_pool(name="work", bufs=2) as kp, \
         tc.tile_pool(name="seg", bufs=3) as sp, \
         tc.tile_pool(name="ps_a", bufs=4, space="PSUM") as pa, \
         tc.tile_pool(name="ps_rep", bufs=2, space="PSUM") as prep, \
         tc.tile_pool(name="ps_y", bufs=2, space="PSUM") as py:

        # ---- constant weights ----
        w_inT = wp.tile([DM, 2 * DIN], F32)
        w_effT = wp.tile([DIN, DIN], F32)
        w_bcT = wp.tile([DIN, 2 * N], F32)
        a_cols = wp.tile([DIN, N], F32)
        conv_w = wp.tile([DIN, K], F32)
        conv_b = wp.tile([DIN, 1], F32)
        conv_bh = wp.tile([DIN, 1], F32)
        b_dt = wp.tile([DIN, 1], F32)
        d_col = wp.tile([DIN, 1], F32)
        w1T = wp.tile([DIN, HID], F32)
        b_c1 = wp.tile([HID, 1], F32)
        w2T = wp.tile([HID + 1, NL], F32)
        ident = wp.tile([128, 128], F32)
        e_sel = wp.tile([2 * N, 2 * N * DIN], F32)
        ones_row = wp.tile([1, DIN], F32)
        for t_, d_ in [(w_inT, w_inT_d), (w_effT, w_effT_d), (w_bcT, w_bcT_d),
                       (a_cols, a_cols_d), (conv_w, conv_w_d), (conv_b, conv_b_d),
                       (conv_bh, conv_bh_d),
                       (b_dt, b_dt_d), (d_col, d_col_d), (w1T, w1T_d),
                       (w2T, w2T_d), (ident, ident_d), (e_sel, e_sel_d),
                       (b_c1, b_c1_d)]:
            nc.sync.dma_start(t_[:], d_[:])
        nc.vector.memset(ones_row[:], 1.0)

        # persistent state (per batch element, reset at chunk 0)
        h_carry = wp.tile([DIN, N], F32)
        halo = wp.tile([DIN, K - 1], F32)

        for b in range(BLOC):
            for ch in range(NCH):
                t0 = ch * C
                # ---- load x chunk [C, DM] as [128, Q*DM] ----
                x_in = kp.tile([128, Q * DM], F32)
                src = x_d[b, t0:t0 + C, :].rearrange("(q p) d -> p q d", p=128)
                nc.sync.dma_start(x_in[:].rearrange("p (q d) -> p q d", q=Q), src)

                # ---- transpose to xT [DM, C] ----
                xT_ps = pa.tile([DM, C], F32, tag="a")
                for q in range(Q):
                    nc.tensor.transpose(
                        xT_ps[:, q * 128:(q + 1) * 128],
                        x_in[:, q * DM:(q + 1) * DM], ident[:])
                xT = kp.tile([DM, C], F32)
                nc.scalar.copy(xT[:], xT_ps[:])

                # ---- input projection ----
                xi_ps = pa.tile([DIN, C], F32, tag="a")
                z_ps = pa.tile([DIN, C], F32, tag="a")
                nc.tensor.matmul(xi_ps[:], w_inT[:, 0:DIN], xT[:],
                                 start=True, stop=True)
                nc.tensor.matmul(z_ps[:], w_inT[:, DIN:2 * DIN], xT[:],
                                 start=True, stop=True)

                # ---- causal depthwise conv (K=4) + silu ----
                # out[t] = sum_k w[k] * xi[t-3+k];  halo holds xi[-3:]
                xc_pre = kp.tile([DIN, C], F32)
                # tap k=3 covers all columns
                nc.vector.tensor_scalar(xc_pre[:], xi_ps[:], conv_w[:, 3:4],
                                        None, op0=OP.mult)
                for k in range(3):  # taps 0..2, shifted
                    sh = 3 - k
                    nc.vector.scalar_tensor_tensor(
                        xc_pre[:, sh:C], xi_ps[:, 0:C - sh], conv_w[:, k:k + 1],
                        xc_pre[:, sh:C], op0=OP.mult, op1=OP.add)
                    if ch == 0:
                        pass  # halo is zero at sequence start -> contribution 0
                    else:
                        nc.vector.scalar_tensor_tensor(
                            xc_pre[:, 0:sh], halo[:, k:3], conv_w[:, k:k + 1],
                            xc_pre[:, 0:sh], op0=OP.mult, op1=OP.add)
                if ch == 0:
                    # still need tap contributions for t<sh from zero halo: none
                    pass
                # update halo for next chunk (before silu; reads xi_ps)
                if ch < NCH - 1:
                    nc.vector.tensor_copy(halo[:], xi_ps[:, C - 3:C])
                # silu(v) = v * (0.5 + 0.5*tanh(v/2)), v = xc_pre + conv_b
                th = kp.tile([DIN, C], F32)
                nc.scalar.activation(th[:], xc_pre[:], ACTF.Tanh,
                                     bias=conv_bh[:], scale=0.5)
                xcb = kp.tile([DIN, C], F32)
                nc.vector.tensor_scalar(xcb[:], xc_pre[:], conv_b[:], None,
                                        op0=OP.add)
                sg = kp.tile([DIN, C], F32)
                nc.vector.tensor_scalar(sg[:], th[:], 0.5, 0.5,
                                        op0=OP.mult, op1=OP.add)
                xc = kp.tile([DIN, C], F32)
                nc.vector.tensor_tensor(xc[:], xcb[:], sg[:], op=OP.mult)

                # ---- x_proj: delta / B / C ----
                dpre_ps = pa.tile([DIN, C], F32, tag="a")
                nc.tensor.matmul(dpre_ps[:], w_effT[:], xc[:], start=True, stop=True)
                # softplus(v) = ln(exp(v) + 1), v = dpre + b_dt
                e_sp = kp.tile([DIN, C], F32)
                nc.scalar.activation(e_sp[:], dpre_ps[:], ACTF.Exp, bias=b_dt[:])
                delta = kp.tile([DIN, C], F32)
                nc.scalar.activation(delta[:], e_sp[:], ACTF.Ln, bias=1.0)
                bc_ps = pa.tile([2 * N, C], F32, tag="a")
                nc.tensor.matmul(bc_ps[:], w_bcT[:], xc[:], start=True, stop=True)
                bc_sb = kp.tile([2 * N, C], F32)
                nc.scalar.copy(bc_sb[:], bc_ps[:])

                # u = delta * xc
                u = kp.tile([DIN, C], F32)
                nc.vector.tensor_tensor(u[:], delta[:], xc[:], op=OP.mult)

                # silu(z) via tanh
                th_z = kp.tile([DIN, C], F32)
                nc.scalar.activation(th_z[:], z_ps[:], ACTF.Tanh, scale=0.5)
                sg_z = kp.tile([DIN, C], F32)
                nc.vector.tensor_scalar(sg_z[:], th_z[:], 0.5, 0.5,
                                        op0=OP.mult, op1=OP.add)
                zs = kp.tile([DIN, C], F32)
                nc.vector.tensor_tensor(zs[:], z_ps[:], sg_z[:], op=OP.mult)

                # ---- per-state-index scan ----
                y_ps = py.tile([DIN, C], F32, tag="y")
                for n in range(N):
                    dA = sp.tile([DIN, C], F32, tag="dA")
                    nc.scalar.activation(dA[:], delta[:], ACTF.Exp,
                                         scale=a_cols[:, n:n + 1])
                    brep = prep.tile([DIN, C], F32, tag="rep")
                    nc.tensor.matmul(brep[:],
                                     e_sel[:, n * DIN:(n + 1) * DIN],
                                     bc_sb[:], start=True, stop=True)
                    dBx = sp.tile([DIN, C], F32, tag="dBx")
                    nc.vector.tensor_tensor(dBx[:], brep[:], u[:], op=OP.mult)
                    h = sp.tile([DIN, C], F32, tag="h")
                    init = 0.0 if ch == 0 else h_carry[:, n:n + 1]
                    nc.vector.tensor_tensor_scan(h[:], dA[:], dBx[:], init,
                                                 op0=OP.mult, op1=OP.add)
                    if ch < NCH - 1:
                        nc.vector.tensor_copy(h_carry[:, n:n + 1], h[:, C - 1:C])
                    crep = prep.tile([DIN, C], F32, tag="rep")
                    nc.tensor.matmul(crep[:],
                                     e_sel[:, (N + n) * DIN:(N + n + 1) * DIN],
                                     bc_sb[:], start=True, stop=True)
                    hC = sp.tile([DIN, C], F32, tag="hC")
                    nc.vector.tensor_tensor(hC[:], crep[:], h[:], op=OP.mult)
                    nc.tensor.matmul(y_ps[:], ident[0:DIN, 0:DIN], hC[:],
                                     start=(n == 0), stop=(n == N - 1))

                # ---- gate + output ----
                y1 = kp.tile([DIN, C], F32)
                nc.vector.scalar_tensor_tensor(y1[:], xc[:], d_col[:], y_ps[:],
                                               op0=OP.mult, op1=OP.add)
                y_gated = kp.tile([DIN, C], F32)
                nc.vector.tensor_tensor(y_gated[:], y1[:], zs[:], op=OP.mult)

                g_ps = pa.tile([HID, C], F32, tag="a")
                nc.tensor.matmul(g_ps[:], w1T[:], y_gated[:], start=True, stop=True)
                g_aug = kp.tile([HID + 1, C], F32)
                nc.scalar.activation(g_aug[0:HID, :], g_ps[:], ACTF.Relu,
                                     bias=b_c1[:])
                nc.vector.memset(g_aug[HID:HID + 1, :], 1.0)

                out_sb = kp.tile([128, Q * NL], F32)
                for q in range(Q):
                    lg_ps = pa.tile([128, NL], F32, tag="a")
                    nc.tensor.matmul(lg_ps[:], g_aug[:, q * 128:(q + 1) * 128],
                                     w2T[:], start=True, stop=True)
                    nc.scalar.copy(out_sb[:, q * NL:(q + 1) * NL], lg_ps[:])
                dst = out_d[b, t0:t0 + C, :].rearrange("(q p) c -> p q c", p=128)
                nc.sync.dma_start(
                    dst, out_sb[:].rearrange("p (q c) -> p q c", q=Q))

    nc.compile()
    return nc


def _e_sel():
    # e_sel[:, j*DIN:(j+1)*DIN] is [2N, DIN]; row j all-ones, rest zero:
    # lhsT for the TensorE partition-broadcast of bc row j.
    e = np.zeros((2 * N, 2 * N * DIN), np.float32)
    for j in range(2 * N):
        e[j, j * DIN:(j + 1) * DIN] = 1.0
    return e


def _prep_inputs(inputs):
    x = np.ascontiguousarray(inputs["x"], dtype=np.float32)
    W_in = np.asarray(inputs["W_in"], np.float64)
    conv_w = np.asarray(inputs["conv_w"], np.float64)
    conv_b = np.asarray(inputs["conv_b"], np.float64)
    W_xproj = np.asarray(inputs["W_xproj"], np.float64)
    W_dt = np.asarray(inputs["W_dt"], np.float64)
    b_dt = np.asarray(inputs["b_dt"], np.float64)
    A_log = np.asarray(inputs["A_log"], np.float64)
    D = np.asarray(inputs["D"], np.float64)
    W_out = np.asarray(inputs["W_out"], np.float64)
    W_c1 = np.asarray(inputs["W_c1"], np.float64)
    b_c1 = np.asarray(inputs["b_c1"], np.float64)
    W_c2 = np.asarray(inputs["W_c2"], np.float64)
    b_c2 = np.asarray(inputs["b_c2"], np.float64)

    f = lambda a: np.ascontiguousarray(a, dtype=np.float32)
    shared = {
        "w_inT": f(W_in.T),
        "w_effT": f((W_dt @ W_xproj[:DTR]).T),
        "w_bcT": f(W_xproj[DTR:].T),
        "a_cols": f(-np.exp(A_log)),
        "conv_w": f(conv_w),
        "conv_b": f(conv_b[:, None]),
        "conv_bh": f(conv_b[:, None] * 0.5),
        "b_dt": f(b_dt[:, None]),
        "d_col": f(D[:, None]),
        "w1T": f((W_c1 @ W_out).T),
        "b_c1": f(b_c1[:, None]),
        "w2T": f(np.vstack([W_c2.T, b_c2[None, :]])),
        "ident": np.eye(128, dtype=np.float32),
        "e_sel": _e_sel(),
    }
    in_maps = []
    for c in range(NCORES):
        m = dict(shared)
        m["x"] = x[c * BLOC:(c + 1) * BLOC]
        in_maps.append(m)
    return in_maps


def kernel(**inputs):
    return _run(inputs, trace=False)[0]


def kernel_traced(**inputs):
    return _run(inputs, trace=True)


def _run(inputs, trace=False):
    key = "nc"
    if key not in _cache:
        _cache[key] = _build({})
    nc = _cache[key]
    in_maps = _prep_inputs(inputs)
    res = run_bass_kernel_spmd(nc, in_maps, core_ids=list(range(NCORES)),
                               trace=trace)
    out = np.concatenate([r["out"] for r in res.results], axis=0)
    return out, res


# revision 18
# speedup vs baseline: 1.6847x; 1.6847x over previous
"""Trainium2 Bass kernel for nn_Network_61658550501610 (Mamba block + MLP head).

Reference computation (per batch element b, sequence length L=2048):
  xz = x @ W_in.T; xi, z = split(xz)
  xc = silu(causal_depthwise_conv(xi, conv_w) + conv_b)
  x_dbl = xc @ W_xproj.T -> (dt, B, C)
  delta = softplus(dt @ W_dt.T + b_dt)
  h_t = exp(delta*A)*h_{t-1} + delta*B*xc   (selective scan, state [82,16])
  y = (h @ C) + D*xc; y *= silu(z)
  out = y @ W_out.T;  logits = relu(out@W_c1.T+b_c1)@W_c2.T + b_c2

Sharding: data-parallel over batch (B=16 -> 2 per core across 8 cores).

Layout on chip: d_inner (82) on partitions, time on free dim. The scan uses
the DVE tensor_tensor_scan instruction per state index n (16 of them), with
chunk carries through per-partition initial values. B[n,:]/C[n,:] are
broadcast across partitions with TensorE ones-matmuls into PSUM; the sum
over n runs as accumulating identity matmuls on TensorE.
"""
import ml_dtypes
import numpy as np

import concourse.bacc as bacc
import concourse.tile as tile
import concourse.mybir as mybir
from concourse.bass_utils import run_bass_kernel_spmd

F32 = mybir.dt.float32
BF16 = mybir.dt.bfloat16
OP = mybir.AluOpType
ACTF = mybir.ActivationFunctionType
AX = mybir.AxisListType

# problem dims (hardcoded per contract)
B, L, DM = 16, 2048, 41
DIN, N, K = 82, 16, 4          # d_inner, d_state, d_conv
DTR, HID, NL = 3, 64, 10
NCORES = 8
BLOC = B // NCORES             # batch per core

C = 512                        # time-chunk length
NCH = L // C                   # chunks per batch element
Q = C // 128                   # 128-row subtiles per chunk

_cache = {}


def _build(cfg):
    nc = bacc.Bacc("TRN2", target_bir_lowering=False, debug=False,
                   enable_asserts=False)

    def din(name, shape):
        return nc.dram_tensor(name, list(shape), F32, kind="ExternalInput").ap()

    x_d = din("x", (BLOC, L, DM))
    w_inT_d = din("w_inT", (DM, 2 * DIN))
    w_effT_d = din("w_effT", (DIN, DIN))
    w_bcT_d = din("w_bcT", (DIN, 2 * N))
    a_cols_d = din("a_cols", (DIN, N))
    conv_w_d = din("conv_w", (DIN, K))
    conv_b_d = din("conv_b", (DIN, 1))
    conv_bh_d = din("conv_bh", (DIN, 1))
    b_dt_d = din("b_dt", (DIN, 1))
    d_col_d = din("d_col", (DIN, 1))
    w1T_d = din("w1T", (DIN, HID))
    b_c1_d = din("b_c1", (HID, 1))
    w2T_d = din("w2T", (HID + 1, NL))
    ident_d = din("ident", (128, 128))
    identb_d = nc.dram_tensor("identb", [DIN, DIN], BF16, kind="ExternalInput").ap()
    e_sel_d = nc.dram_tensor("e_sel", [2 * N, 2 * N * DIN], BF16,
                             kind="ExternalInput").ap()
    out_d = nc.dram_tensor("out", [BLOC, L, NL], F32, kind="ExternalOutput").ap()

    with tile.TileContext(nc) as tc, tc.tile_pool(name="wts", bufs=1) as wp, \
         tc.tile_pool(name="work", bufs=2) as kp, \
         tc.tile_pool(name="seg", bufs=3) as sp, \
         tc.tile_pool(name="ps_a", bufs=4, space="PSUM") as pa, \
         tc.tile_pool(name="ps_rep", bufs=2, space="PSUM") as prep, \
         tc.tile_pool(name="ps_y", bufs=2, space="PSUM") as py:

        # ---- constant weights ----
        w_inT = wp.tile([DM, 2 * DIN], F32)
        w_effT = wp.tile([DIN, DIN], F32)
        w_bcT = wp.tile([DIN, 2 * N], F32)
        a_cols = wp.tile([DIN, N], F32)
        conv_w = wp.tile([DIN, K], F32)
        conv_b = wp.tile([DIN, 1], F32)
        conv_bh = wp.tile([DIN, 1], F32)
        b_dt = wp.tile([DIN, 1], F32)
        d_col = wp.tile([DIN, 1], F32)
        w1T = wp.tile([DIN, HID], F32)
        b_c1 = wp.tile([HID, 1], F32)
        w2T = wp.tile([HID + 1, NL], F32)
        ident = wp.tile([128, 128], F32)
        identb = wp.tile([DIN, DIN], BF16)
        e_sel = wp.tile([2 * N, 2 * N * DIN], BF16)
        ones_row = wp.tile([1, DIN], F32)
        for t_, d_ in [(w_inT, w_inT_d), (w_effT, w_effT_d), (w_bcT, w_bcT_d),
                       (a_cols, a_cols_d), (conv_w, conv_w_d), (conv_b, conv_b_d),
                       (conv_bh, conv_bh_d),
                       (b_dt, b_dt_d), (d_col, d_col_d), (w1T, w1T_d),
                       (w2T, w2T_d), (ident, ident_d), (e_sel, e_sel_d),
                       (identb, identb_d),
                       (b_c1, b_c1_d)]:
            nc.sync.dma_start(t_[:], d_[:])
        nc.vector.memset(ones_row[:], 1.0)

        # persistent state (per batch element, reset at chunk 0)
        h_carry = wp.tile([DIN, N], F32)
        halo = wp.tile([DIN, K - 1], F32)

        for b in range(BLOC):
            for ch in range(NCH):
                t0 = ch * C
                # ---- load x chunk [C, DM] as [128, Q*DM] ----
                x_in = kp.tile([128, Q * DM], F32)
                src = x_d[b, t0:t0 + C, :].rearrange("(q p) d -> p q d", p=128)
                nc.sync.dma_start(x_in[:].rearrange("p (q d) -> p q d", q=Q), src)

                # ---- transpose to xT [DM, C] ----
                xT_ps = pa.tile([DM, C], F32, tag="a")
                for q in range(Q):
                    nc.tensor.transpose(
                        xT_ps[:, q * 128:(q + 1) * 128],
                        x_in[:, q * DM:(q + 1) * DM], ident[:])
                xT = kp.tile([DM, C], F32)
                nc.scalar.copy(xT[:], xT_ps[:])

                # ---- input projection ----
                xi_ps = pa.tile([DIN, C], F32, tag="a")
                z_ps = pa.tile([DIN, C], F32, tag="a")
                nc.tensor.matmul(xi_ps[:], w_inT[:, 0:DIN], xT[:],
                                 start=True, stop=True)
                nc.tensor.matmul(z_ps[:], w_inT[:, DIN:2 * DIN], xT[:],
                                 start=True, stop=True)

                # ---- causal depthwise conv (K=4) + silu ----
                # out[t] = sum_k w[k] * xi[t-3+k];  halo holds xi[-3:]
                xc_pre = kp.tile([DIN, C], F32)
                # tap k=3 covers all columns
                nc.vector.tensor_scalar(xc_pre[:], xi_ps[:], conv_w[:, 3:4],
                                        None, op0=OP.mult)
                for k in range(3):  # taps 0..2, shifted
                    sh = 3 - k
                    nc.vector.scalar_tensor_tensor(
                        xc_pre[:, sh:C], xi_ps[:, 0:C - sh], conv_w[:, k:k + 1],
                        xc_pre[:, sh:C], op0=OP.mult, op1=OP.add)
                    if ch == 0:
                        pass  # halo is zero at sequence start -> contribution 0
                    else:
                        nc.vector.scalar_tensor_tensor(
                            xc_pre[:, 0:sh], halo[:, k:3], conv_w[:, k:k + 1],
                            xc_pre[:, 0:sh], op0=OP.mult, op1=OP.add)
                if ch == 0:
                    # still need tap contributions for t<sh from zero halo: none
                    pass
                # update halo for next chunk (before silu; reads xi_ps)
                if ch < NCH - 1:
                    nc.vector.tensor_copy(halo[:], xi_ps[:, C - 3:C])
                # silu(v) = v * (0.5 + 0.5*tanh(v/2)), v = xc_pre + conv_b
                th = kp.tile([DIN, C], F32)
                nc.scalar.activation(th[:], xc_pre[:], ACTF.Tanh,
                                     bias=conv_bh[:], scale=0.5)
                xcb = kp.tile([DIN, C], F32)
                nc.vector.tensor_scalar(xcb[:], xc_pre[:], conv_b[:], None,
                                        op0=OP.add)
                sg = kp.tile([DIN, C], F32)
                nc.vector.tensor_scalar(sg[:], th[:], 0.5, 0.5,
                                        op0=OP.mult, op1=OP.add)
                xc = kp.tile([DIN, C], F32)
                nc.vector.tensor_tensor(xc[:], xcb[:], sg[:], op=OP.mult)

                # ---- x_proj: delta / B / C ----
                dpre_ps = pa.tile([DIN, C], F32, tag="a")
                nc.tensor.matmul(dpre_ps[:], w_effT[:], xc[:], start=True, stop=True)
                # softplus(v) = ln(exp(v) + 1), v = dpre + b_dt
                e_sp = kp.tile([DIN, C], F32)
                nc.scalar.activation(e_sp[:], dpre_ps[:], ACTF.Exp, bias=b_dt[:])
                delta = kp.tile([DIN, C], F32)
                nc.scalar.activation(delta[:], e_sp[:], ACTF.Ln, bias=1.0)
                bc_ps = pa.tile([2 * N, C], F32, tag="a")
                nc.tensor.matmul(bc_ps[:], w_bcT[:], xc[:], start=True, stop=True)
                bc_sb = kp.tile([2 * N, C], BF16)
                nc.scalar.copy(bc_sb[:], bc_ps[:])

                # u = delta * xc
                u = kp.tile([DIN, C], F32)
                nc.vector.tensor_tensor(u[:], delta[:], xc[:], op=OP.mult)

                # silu(z) via tanh
                th_z = kp.tile([DIN, C], F32)
                nc.scalar.activation(th_z[:], z_ps[:], ACTF.Tanh, scale=0.5)
                sg_z = kp.tile([DIN, C], F32)
                nc.vector.tensor_scalar(sg_z[:], th_z[:], 0.5, 0.5,
                                        op0=OP.mult, op1=OP.add)
                zs = kp.tile([DIN, C], F32)
                nc.vector.tensor_tensor(zs[:], z_ps[:], sg_z[:], op=OP.mult)

                # ---- per-state-index scan ----
                y_ps = py.tile([DIN, C], F32, tag="y")
                for n in range(N):
                    dA = sp.tile([DIN, C], F32, tag="dA")
                    nc.scalar.activation(dA[:], delta[:], ACTF.Exp,
                                         scale=a_cols[:, n:n + 1])
                    brep = prep.tile([DIN, C], F32, tag="rep")
                    nc.tensor.matmul(brep[:],
                                     e_sel[:, n * DIN:(n + 1) * DIN],
                                     bc_sb[:], start=True, stop=True)
                    dBx = sp.tile([DIN, C], F32, tag="dBx")
                    nc.vector.tensor_tensor(dBx[:], brep[:], u[:], op=OP.mult)
                    h = sp.tile([DIN, C], F32, tag="h")
                    init = 0.0 if ch == 0 else h_carry[:, n:n + 1]
                    nc.vector.tensor_tensor_scan(h[:], dA[:], dBx[:], init,
                                                 op0=OP.mult, op1=OP.add)
                    if ch < NCH - 1:
                        nc.vector.tensor_copy(h_carry[:, n:n + 1], h[:, C - 1:C])
                    crep = prep.tile([DIN, C], F32, tag="rep")
                    nc.tensor.matmul(crep[:],
                                     e_sel[:, (N + n) * DIN:(N + n + 1) * DIN],
                                     bc_sb[:], start=True, stop=True)
                    hC = sp.tile([DIN, C], BF16, tag="hC")
                    nc.vector.tensor_tensor(hC[:], crep[:], h[:], op=OP.mult)
                    nc.tensor.matmul(y_ps[:], identb[:], hC[:],
                                     start=(n == 0), stop=(n == N - 1))

                # ---- gate + output ----
                y1 = kp.tile([DIN, C], F32)
                nc.vector.scalar_tensor_tensor(y1[:], xc[:], d_col[:], y_ps[:],
                                               op0=OP.mult, op1=OP.add)
                y_gated = kp.tile([DIN, C], F32)
                nc.vector.tensor_tensor(y_gated[:], y1[:], zs[:], op=OP.mult)

                g_ps = pa.tile([HID, C], F32, tag="a")
                nc.tensor.matmul(g_ps[:], w1T[:], y_gated[:], start=True, stop=True)
                g_aug = kp.tile([HID + 1, C], F32)
                nc.scalar.activation(g_aug[0:HID, :], g_ps[:], ACTF.Relu,
                                     bias=b_c1[:])
                nc.vector.memset(g_aug[HID:HID + 1, :], 1.0)

                out_sb = kp.tile([128, Q * NL], F32)
                for q in range(Q):
                    lg_ps = pa.tile([128, NL], F32, tag="a")
                    nc.tensor.matmul(lg_ps[:], g_aug[:, q * 128:(q + 1) * 128],
                                     w2T[:], start=True, stop=True)
                    nc.scalar.copy(out_sb[:, q * NL:(q + 1) * NL], lg_ps[:])
                dst = out_d[b, t0:t0 + C, :].rearrange("(q p) c -> p q c", p=128)
                nc.sync.dma_start(
                    dst, out_sb[:].rearrange("p (q c) -> p q c", q=Q))

    nc.compile()
    return nc


def _e_sel():
    # e_sel[:, j*DIN:(j+1)*DIN] is [2N, DIN]; row j all-ones, rest zero:
    # lhsT for the TensorE partition-broadcast of bc row j.
    e = np.zeros((2 * N, 2 * N * DIN), np.float32)
    for j in range(2 * N):
        e[j, j * DIN:(j + 1) * DIN] = 1.0
    return e


def _prep_inputs(inputs):
    x = np.ascontiguousarray(inputs["x"], dtype=np.float32)
    W_in = np.asarray(inputs["W_in"], np.float64)
    conv_w = np.asarray(inputs["conv_w"], np.float64)
    conv_b = np.asarray(inputs["conv_b"], np.float64)
    W_xproj = np.asarray(inputs["W_xproj"], np.float64)
    W_dt = np.asarray(inputs["W_dt"], np.float64)
    b_dt = np.asarray(inputs["b_dt"], np.float64)
    A_log = np.asarray(inputs["A_log"], np.float64)
    D = np.asarray(inputs["D"], np.float64)
    W_out = np.asarray(inputs["W_out"], np.float64)
    W_c1 = np.asarray(inputs["W_c1"], np.float64)
    b_c1 = np.asarray(inputs["b_c1"], np.float64)
    W_c2 = np.asarray(inputs["W_c2"], np.float64)
    b_c2 = np.asarray(inputs["b_c2"], np.float64)

    f = lambda a: np.ascontiguousarray(a, dtype=np.float32)
    shared = {
        "w_inT": f(W_in.T),
        "w_effT": f((W_dt @ W_xproj[:DTR]).T),
        "w_bcT": f(W_xproj[DTR:].T),
        "a_cols": f(-np.exp(A_log)),
        "conv_w": f(conv_w),
        "conv_b": f(conv_b[:, None]),
        "conv_bh": f(conv_b[:, None] * 0.5),
        "b_dt": f(b_dt[:, None]),
        "d_col": f(D[:, None]),
        "w1T": f((W_c1 @ W_out).T),
        "b_c1": f(b_c1[:, None]),
        "w2T": f(np.vstack([W_c2.T, b_c2[None, :]])),
        "ident": np.eye(128, dtype=np.float32),
        "identb": np.eye(DIN, dtype=ml_dtypes.bfloat16),
        "e_sel": _e_sel().astype(ml_dtypes.bfloat16),
    }
    in_maps = []
    for c in range(NCORES):
        m = dict(shared)
        m["x"] = x[c * BLOC:(c + 1) * BLOC]
        in_maps.append(m)
    return in_maps


def kernel(**inputs):
    return _run(inputs, trace=False)[0]


def kernel_traced(**inputs):
    return _run(inputs, trace=True)


def _run(inputs, trace=False):
    key = "nc"
    if key not in _cache:
        _cache[key] = _build({})
    nc = _cache[key]
    in_maps = _prep_inputs(inputs)
    res = run_bass_kernel_spmd(nc, in_maps, core_ids=list(range(NCORES)),
                               trace=trace)
    out = np.concatenate([r["out"] for r in res.results], axis=0)
    return out, res


# revision 19
# speedup vs baseline: 2.0103x; 1.1932x over previous
"""Trainium2 Bass kernel for nn_Network_61658550501610 (Mamba block + MLP head).

Reference computation (per batch element b, sequence length L=2048):
  xz = x @ W_in.T; xi, z = split(xz)
  xc = silu(causal_depthwise_conv(xi, conv_w) + conv_b)
  x_dbl = xc @ W_xproj.T -> (dt, B, C)
  delta = softplus(dt @ W_dt.T + b_dt)
  h_t = exp(delta*A)*h_{t-1} + delta*B*xc   (selective scan, state [82,16])
  y = (h @ C) + D*xc; y *= silu(z)
  out = y @ W_out.T;  logits = relu(out@W_c1.T+b_c1)@W_c2.T + b_c2

Sharding: data-parallel over batch (B=16 -> 2 per core across 8 cores).

Layout on chip: d_inner (82) on partitions, time on free dim. The scan uses
the DVE tensor_tensor_scan instruction per state index n (16 of them), with
chunk carries through per-partition initial values. B[n,:]/C[n,:] are
broadcast across partitions with TensorE ones-matmuls into PSUM; the sum
over n runs as accumulating identity matmuls on TensorE.
"""
import ml_dtypes
import numpy as np

import concourse.bacc as bacc
import concourse.tile as tile
import concourse.mybir as mybir
from concourse.bass_utils import run_bass_kernel_spmd

F32 = mybir.dt.float32
BF16 = mybir.dt.bfloat16
OP = mybir.AluOpType
ACTF = mybir.ActivationFunctionType
AX = mybir.AxisListType

# problem dims (hardcoded per contract)
B, L, DM = 16, 2048, 41
DIN, N, K = 82, 16, 4          # d_inner, d_state, d_conv
DTR, HID, NL = 3, 64, 10
NCORES = 8
BLOC = B // NCORES             # batch per core

C = 512                        # time-chunk length
NCH = L // C                   # chunks per batch element
Q = C // 128                   # 128-row subtiles per chunk

_cache = {}


def _build(cfg):
    nc = bacc.Bacc("TRN2", target_bir_lowering=False, debug=False,
                   enable_asserts=False)

    def din(name, shape):
        return nc.dram_tensor(name, list(shape), F32, kind="ExternalInput").ap()

    x_d = din("x", (BLOC, L, DM))
    w_inT_d = din("w_inT", (DM, 2 * DIN))
    w_effT_d = din("w_effT", (DIN, DIN))
    w_bcT_d = din("w_bcT", (DIN, 2 * N))
    a_cols_d = din("a_cols", (DIN, N))
    conv_w_d = din("conv_w", (DIN, K))
    conv_b_d = din("conv_b", (DIN, 1))
    conv_bh_d = din("conv_bh", (DIN, 1))
    b_dt_d = din("b_dt", (DIN, 1))
    d_col_d = din("d_col", (DIN, 1))
    w1T_d = din("w1T", (DIN, HID))
    b_c1_d = din("b_c1", (HID, 1))
    w2T_d = din("w2T", (HID + 1, NL))
    ident_d = din("ident", (128, 128))
    identb_d = nc.dram_tensor("identb", [DIN, DIN], BF16, kind="ExternalInput").ap()
    e_sel_d = nc.dram_tensor("e_sel", [2 * N, 2 * N * DIN], BF16,
                             kind="ExternalInput").ap()
    out_d = nc.dram_tensor("out", [BLOC, L, NL], F32, kind="ExternalOutput").ap()

    with tile.TileContext(nc) as tc, tc.tile_pool(name="wts", bufs=1) as wp, \
         tc.tile_pool(name="work", bufs=2) as kp, \
         tc.tile_pool(name="seg", bufs=4) as sp, \
         tc.tile_pool(name="ps_f", bufs=3, space="PSUM") as pf, \
         tc.tile_pool(name="ps_t", bufs=2, space="PSUM") as pt, \
         tc.tile_pool(name="ps_rep", bufs=2, space="PSUM") as prep, \
         tc.tile_pool(name="ps_y", bufs=1, space="PSUM") as py:

        # ---- constant weights ----
        w_inT = wp.tile([DM, 2 * DIN], F32)
        w_effT = wp.tile([DIN, DIN], F32)
        w_bcT = wp.tile([DIN, 2 * N], F32)
        a_cols = wp.tile([DIN, N], F32)
        conv_w = wp.tile([DIN, K], F32)
        conv_b = wp.tile([DIN, 1], F32)
        conv_bh = wp.tile([DIN, 1], F32)
        b_dt = wp.tile([DIN, 1], F32)
        d_col = wp.tile([DIN, 1], F32)
        w1T = wp.tile([DIN, HID], F32)
        b_c1 = wp.tile([HID, 1], F32)
        w2T = wp.tile([HID + 1, NL], F32)
        ident = wp.tile([128, 128], F32)
        identb = wp.tile([DIN, DIN], BF16)
        e_sel = wp.tile([2 * N, 2 * N * DIN], BF16)
        ones_row = wp.tile([1, DIN], F32)
        for t_, d_ in [(w_inT, w_inT_d), (w_effT, w_effT_d), (w_bcT, w_bcT_d),
                       (a_cols, a_cols_d), (conv_w, conv_w_d), (conv_b, conv_b_d),
                       (conv_bh, conv_bh_d),
                       (b_dt, b_dt_d), (d_col, d_col_d), (w1T, w1T_d),
                       (w2T, w2T_d), (ident, ident_d), (e_sel, e_sel_d),
                       (identb, identb_d),
                       (b_c1, b_c1_d)]:
            nc.sync.dma_start(t_[:], d_[:])
        nc.vector.memset(ones_row[:], 1.0)

        # persistent state (per batch element, reset at chunk 0)
        h_carry = wp.tile([DIN, N], F32)
        halo = wp.tile([DIN, K - 1], F32)

        for b in range(BLOC):
            for ch in range(NCH):
                t0 = ch * C
                # ---- load x chunk [C, DM] as [128, Q*DM] ----
                x_in = kp.tile([128, Q * DM], F32)
                src = x_d[b, t0:t0 + C, :].rearrange("(q p) d -> p q d", p=128)
                nc.sync.dma_start(x_in[:].rearrange("p (q d) -> p q d", q=Q), src)

                # ---- transpose to xT [DM, C] ----
                xT_ps = pf.tile([DM, C], F32, tag="f")
                for q in range(Q):
                    nc.tensor.transpose(
                        xT_ps[:, q * 128:(q + 1) * 128],
                        x_in[:, q * DM:(q + 1) * DM], ident[:])
                xT = kp.tile([DM, C], F32)
                nc.scalar.copy(xT[:], xT_ps[:])

                # ---- input projection ----
                xi_ps = pf.tile([DIN, C], F32, tag="f")
                z_ps = pf.tile([DIN, C], F32, tag="f")
                nc.tensor.matmul(xi_ps[:], w_inT[:, 0:DIN], xT[:],
                                 start=True, stop=True)
                nc.tensor.matmul(z_ps[:], w_inT[:, DIN:2 * DIN], xT[:],
                                 start=True, stop=True)

                # ---- causal depthwise conv (K=4) + silu ----
                # out[t] = sum_k w[k] * xi[t-3+k];  halo holds xi[-3:]
                xc_pre = kp.tile([DIN, C], F32)
                # tap k=3 covers all columns
                nc.vector.tensor_scalar(xc_pre[:], xi_ps[:], conv_w[:, 3:4],
                                        None, op0=OP.mult)
                for k in range(3):  # taps 0..2, shifted
                    sh = 3 - k
                    nc.vector.scalar_tensor_tensor(
                        xc_pre[:, sh:C], xi_ps[:, 0:C - sh], conv_w[:, k:k + 1],
                        xc_pre[:, sh:C], op0=OP.mult, op1=OP.add)
                    if ch == 0:
                        pass  # halo is zero at sequence start -> contribution 0
                    else:
                        nc.vector.scalar_tensor_tensor(
                            xc_pre[:, 0:sh], halo[:, k:3], conv_w[:, k:k + 1],
                            xc_pre[:, 0:sh], op0=OP.mult, op1=OP.add)
                if ch == 0:
                    # still need tap contributions for t<sh from zero halo: none
                    pass
                # update halo for next chunk (before silu; reads xi_ps)
                if ch < NCH - 1:
                    nc.vector.tensor_copy(halo[:], xi_ps[:, C - 3:C])
                # silu(v) = v * (0.5 + 0.5*tanh(v/2)), v = xc_pre + conv_b
                th = kp.tile([DIN, C], F32)
                nc.scalar.activation(th[:], xc_pre[:], ACTF.Tanh,
                                     bias=conv_bh[:], scale=0.5)
                xcb = kp.tile([DIN, C], F32)
                nc.vector.tensor_scalar(xcb[:], xc_pre[:], conv_b[:], None,
                                        op0=OP.add)
                sg = kp.tile([DIN, C], F32)
                nc.vector.tensor_scalar(sg[:], th[:], 0.5, 0.5,
                                        op0=OP.mult, op1=OP.add)
                xc = kp.tile([DIN, C], F32)
                nc.vector.tensor_tensor(xc[:], xcb[:], sg[:], op=OP.mult)

                # ---- x_proj: delta / B / C ----
                dpre_ps = pf.tile([DIN, C], F32, tag="f")
                nc.tensor.matmul(dpre_ps[:], w_effT[:], xc[:], start=True, stop=True)
                # softplus(v) = ln(exp(v) + 1), v = dpre + b_dt
                e_sp = kp.tile([DIN, C], F32)
                nc.scalar.activation(e_sp[:], dpre_ps[:], ACTF.Exp, bias=b_dt[:])
                delta = kp.tile([DIN, C], F32)
                nc.scalar.activation(delta[:], e_sp[:], ACTF.Ln, bias=1.0)
                bc_ps = pf.tile([2 * N, C], F32, tag="f")
                nc.tensor.matmul(bc_ps[:], w_bcT[:], xc[:], start=True, stop=True)
                bc_sb = kp.tile([2 * N, C], BF16)
                nc.scalar.copy(bc_sb[:], bc_ps[:])

                # u = delta * xc
                u = kp.tile([DIN, C], F32)
                nc.vector.tensor_tensor(u[:], delta[:], xc[:], op=OP.mult)

                # silu(z) via tanh
                th_z = kp.tile([DIN, C], F32)
                nc.scalar.activation(th_z[:], z_ps[:], ACTF.Tanh, scale=0.5)
                sg_z = kp.tile([DIN, C], F32)
                nc.vector.tensor_scalar(sg_z[:], th_z[:], 0.5, 0.5,
                                        op0=OP.mult, op1=OP.add)
                zs = kp.tile([DIN, C], F32)
                nc.vector.tensor_tensor(zs[:], z_ps[:], sg_z[:], op=OP.mult)

                # ---- per-state-index scan ----
                y_ps = py.tile([DIN, C], F32, tag="y")
                for n in range(N):
                    dA = sp.tile([DIN, C], F32, tag="dA")
                    nc.scalar.activation(dA[:], delta[:], ACTF.Exp,
                                         scale=a_cols[:, n:n + 1])
                    brep = prep.tile([DIN, C], F32, tag="rep")
                    nc.tensor.matmul(brep[:],
                                     e_sel[:, n * DIN:(n + 1) * DIN],
                                     bc_sb[:], start=True, stop=True)
                    dBx = sp.tile([DIN, C], F32, tag="dBx")
                    nc.vector.tensor_tensor(dBx[:], brep[:], u[:], op=OP.mult)
                    h = sp.tile([DIN, C], F32, tag="h")
                    init = 0.0 if ch == 0 else h_carry[:, n:n + 1]
                    nc.vector.tensor_tensor_scan(h[:], dA[:], dBx[:], init,
                                                 op0=OP.mult, op1=OP.add)
                    if ch < NCH - 1:
                        nc.vector.tensor_copy(h_carry[:, n:n + 1], h[:, C - 1:C])
                    crep = prep.tile([DIN, C], F32, tag="rep")
                    nc.tensor.matmul(crep[:],
                                     e_sel[:, (N + n) * DIN:(N + n + 1) * DIN],
                                     bc_sb[:], start=True, stop=True)
                    hC = sp.tile([DIN, C], BF16, tag="hC")
                    nc.vector.tensor_tensor(hC[:], crep[:], h[:], op=OP.mult)
                    nc.tensor.matmul(y_ps[:], identb[:], hC[:],
                                     start=(n == 0), stop=(n == N - 1))

                # ---- gate + output ----
                y1 = kp.tile([DIN, C], F32)
                nc.vector.scalar_tensor_tensor(y1[:], xc[:], d_col[:], y_ps[:],
                                               op0=OP.mult, op1=OP.add)
                y_gated = kp.tile([DIN, C], F32)
                nc.vector.tensor_tensor(y_gated[:], y1[:], zs[:], op=OP.mult)

                g_ps = pt.tile([HID, C], F32, tag="t")
                nc.tensor.matmul(g_ps[:], w1T[:], y_gated[:], start=True, stop=True)
                g_aug = kp.tile([HID + 1, C], F32)
                nc.scalar.activation(g_aug[0:HID, :], g_ps[:], ACTF.Relu,
                                     bias=b_c1[:])
                nc.vector.memset(g_aug[HID:HID + 1, :], 1.0)

                out_sb = kp.tile([128, Q * NL], F32)
                for q in range(Q):
                    lg_ps = pt.tile([128, NL], F32, tag="t")
                    nc.tensor.matmul(lg_ps[:], g_aug[:, q * 128:(q + 1) * 128],
                                     w2T[:], start=True, stop=True)
                    nc.scalar.copy(out_sb[:, q * NL:(q + 1) * NL], lg_ps[:])
                dst = out_d[b, t0:t0 + C, :].rearrange("(q p) c -> p q c", p=128)
                nc.sync.dma_start(
                    dst, out_sb[:].rearrange("p (q c) -> p q c", q=Q))

    nc.compile()
    return nc


def _e_sel():
    # e_sel[:, j*DIN:(j+1)*DIN] is [2N, DIN]; row j all-ones, rest zero:
    # lhsT for the TensorE partition-broadcast of bc row j.
    e = np.zeros((2 * N, 2 * N * DIN), np.float32)
    for j in range(2 * N):
        e[j, j * DIN:(j + 1) * DIN] = 1.0
    return e


def _prep_inputs(inputs):
    x = np.ascontiguousarray(inputs["x"], dtype=np.float32)
    W_in = np.asarray(inputs["W_in"], np.float64)
    conv_w = np.asarray(inputs["conv_w"], np.float64)
    conv_b = np.asarray(inputs["conv_b"], np.float64)
    W_xproj = np.asarray(inputs["W_xproj"], np.float64)
    W_dt = np.asarray(inputs["W_dt"], np.float64)
    b_dt = np.asarray(inputs["b_dt"], np.float64)
    A_log = np.asarray(inputs["A_log"], np.float64)
    D = np.asarray(inputs["D"], np.float64)
    W_out = np.asarray(inputs["W_out"], np.float64)
    W_c1 = np.asarray(inputs["W_c1"], np.float64)
    b_c1 = np.asarray(inputs["b_c1"], np.float64)
    W_c2 = np.asarray(inputs["W_c2"], np.float64)
    b_c2 = np.asarray(inputs["b_c2"], np.float64)

    f = lambda a: np.ascontiguousarray(a, dtype=np.float32)
    shared = {
        "w_inT": f(W_in.T),
        "w_effT": f((W_dt @ W_xproj[:DTR]).T),
        "w_bcT": f(W_xproj[DTR:].T),
        "a_cols": f(-np.exp(A_log)),
        "conv_w": f(conv_w),
        "conv_b": f(conv_b[:, None]),
        "conv_bh": f(conv_b[:, None] * 0.5),
        "b_dt": f(b_dt[:, None]),
        "d_col": f(D[:, None]),
        "w1T": f((W_c1 @ W_out).T),
        "b_c1": f(b_c1[:, None]),
        "w2T": f(np.vstack([W_c2.T, b_c2[None, :]])),
        "ident": np.eye(128, dtype=np.float32),
        "identb": np.eye(DIN, dtype=ml_dtypes.bfloat16),
        "e_sel": _e_sel().astype(ml_dtypes.bfloat16),
    }
    in_maps = []
    for c in range(NCORES):
        m = dict(shared)
        m["x"] = x[c * BLOC:(c + 1) * BLOC]
        in_maps.append(m)
    return in_maps


def kernel(**inputs):
    return _run(inputs, trace=False)[0]


def kernel_traced(**inputs):
    return _run(inputs, trace=True)


def _run(inputs, trace=False):
    key = "nc"
    if key not in _cache:
        _cache[key] = _build({})
    nc = _cache[key]
    in_maps = _prep_inputs(inputs)
    res = run_bass_kernel_spmd(nc, in_maps, core_ids=list(range(NCORES)),
                               trace=trace)
    out = np.concatenate([r["out"] for r in res.results], axis=0)
    return out, res
